# revision 21
# baseline (speedup 1.0000x reference)
"""HSpatialHyperGCN Trainium2 kernel (v2).

Shapes (hardcoded): x (4, 64, 64, 64); N = 4096 nodes per batch; 4 heads x 64
inter channels; top-5 cosine-similarity hypergraph; uniform degree 6 Laplacian;
hydra attention (global kv); 1x1-conv + folded-BN chain.

Sharding: 8 cores = 4 batches x 2 node-halves (core = 2*b + half).

v2 design vs v1 baseline (627 us):
  - all big matmuls in float32r (1 cycle/row vs 4 for fp32, ~1.5e-4 rel err)
  - sim rows scanned with MAX8/FIND_INDEX8 *directly on PSUM* in two
    2048-column halves (no PSUM->SBUF copy of the 8.4M-entry sim matrix)
  - halves merged with a bit-pack trick: (value & 0xFFFFF000) | column_index,
    max8 over the 16 packed candidates, AND-decode -> exact-ish top-5 with
    index tie-break, no duplicate-index pathology
  - pack/decode on gpsimd (feeds its own indirect gathers), scan on DVE,
    table normalization on scalar, conv chains on PE: engines balanced so the
    irreducible DVE scan (~140 us) is the critical path
  - k*v Laplacian product accumulated on DVE in fp16; kv reduced via a
    (1/36)-weighted ones-matmul; AllReduce over batch pairs; kv folded into wp
  - lhsT of sim = raw x rows (row scale cannot change a row's top-k)
"""

import sys

sys.path.insert(0, "/opt/trn_rl_repo")

import numpy as np

from concourse import bass, mybir, tile, bacc
from concourse.bass_utils import run_bass_kernel_spmd

F32 = mybir.dt.float32
F32R = mybir.dt.float32r
F16 = mybir.dt.float16
U32 = mybir.dt.uint32
AF = mybir.ActivationFunctionType
ALU = mybir.AluOpType
AXX = mybir.AxisListType.X

B, C, H, W = 4, 64, 64, 64
N = H * W            # 4096
NH = 4
INTER = 64
OC = NH * INTER      # 256
K = 5
ROWS = N // 2        # 2048 rows per core
NT = ROWS // 128     # 16 row tiles per core
BN_EPS = 1e-5

_CACHE = {}


def _build_bass(collective=True):
    nc = bacc.Bacc(None, target_bir_lowering=False, debug=False, num_devices=8)

    xa = nc.dram_tensor("xa", [C + 1, N], F32R, kind="ExternalInput")
    xr = nc.dram_tensor("xr", [C + 1, ROWS], F32R, kind="ExternalInput")
    wkvq = nc.dram_tensor("wkvq", [C + 1, 3 * OC], F32R, kind="ExternalInput")
    wpt = nc.dram_tensor("wpt", [128, 2, 64], F32, kind="ExternalInput")
    w1t = nc.dram_tensor("w1t", [64, 64], F16, kind="ExternalInput")
    w2t = nc.dram_tensor("w2t", [64, 64], F16, kind="ExternalInput")
    b1ff = nc.dram_tensor("b1ff", [64, 1], F32, kind="ExternalInput")
    b2f = nc.dram_tensor("b2f", [64, 1], F32, kind="ExternalInput")
    one64 = nc.dram_tensor("one64", [1, 64], F32R, kind="ExternalInput")
    i128h = nc.dram_tensor("i128h", [128, 128], F16, kind="ExternalInput")
    rrow_in = nc.dram_tensor("rrow_in", [1, N], F32R, kind="ExternalInput")
    c36 = nc.dram_tensor("c36", [128, 1], F32, kind="ExternalInput")

    out_half = nc.dram_tensor("out_half", [64, ROWS], F32, kind="ExternalOutput")

    ktable = nc.dram_tensor("ktable", [N, 2 * OC], F16)  # internal per-core DRAM

    with tile.TileContext(nc) as tc:
        with (
            tc.tile_pool(name="const", bufs=1) as cp,
            tc.tile_pool(name="work", bufs=6) as wp_,
            tc.tile_pool(name="tabp", bufs=4) as tabp,
            tc.tile_pool(name="gp", bufs=4) as gp,
            tc.tile_pool(name="ep", bufs=2) as ep,
            tc.tile_pool(name="dram", bufs=2, space="DRAM") as dp,
        ):
            # ---- persistent loads
            xa_t = cp.tile([C + 1, N], F32R)
            for j in range(8):
                eng = nc.sync if j % 2 == 0 else nc.scalar
                eng.dma_start(out=xa_t[:, j * 512:(j + 1) * 512],
                              in_=xa[:, j * 512:(j + 1) * 512])
            xr_t = cp.tile([C + 1, ROWS], F32R)
            for j in range(4):
                eng = nc.sync if j % 2 == 0 else nc.scalar
                eng.dma_start(out=xr_t[:, j * 512:(j + 1) * 512],
                              in_=xr[:, j * 512:(j + 1) * 512])
            wkvq_t = cp.tile([C + 1, 3 * OC], F32R)
            for j in range(2):
                eng = nc.sync if j % 2 == 0 else nc.scalar
                eng.dma_start(out=wkvq_t[:, j * 384:(j + 1) * 384],
                              in_=wkvq[:, j * 384:(j + 1) * 384])
            wpt_t = cp.tile([128, 2, 64], F32)
            nc.sync.dma_start(out=wpt_t[:], in_=wpt[:])
            w1t_t = cp.tile([64, 64], F16)
            nc.sync.dma_start(out=w1t_t[:], in_=w1t[:])
            w2t_t = cp.tile([64, 64], F16)
            nc.sync.dma_start(out=w2t_t[:], in_=w2t[:])
            b1ff_t = cp.tile([64, 1], F32)
            nc.sync.dma_start(out=b1ff_t[:], in_=b1ff[:])
            b2f_t = cp.tile([64, 1], F32)
            nc.sync.dma_start(out=b2f_t[:], in_=b2f[:])
            one64_t = cp.tile([1, 64], F32R)
            nc.sync.dma_start(out=one64_t[:], in_=one64[:])
            rrow = cp.tile([1, N], F32R)
            nc.scalar.dma_start(out=rrow[:], in_=rrow_in[:])
            c36_t = cp.tile([128, 1], F32)
            nc.sync.dma_start(out=c36_t[:], in_=c36[:])
            i128h_t = cp.tile([128, 128], F16)
            nc.sync.dma_start(out=i128h_t[:], in_=i128h[:])

            xn = cp.tile([C, N], F32R)
            qn_own = cp.tile([128, NT, OC], F16)
            qc0 = cp.tile([128, ROWS], F16)
            qc1 = cp.tile([128, ROWS], F16)
            idx_all = cp.tile([128, NT * 8], U32)
            acc = cp.tile([128, OC], F32)
            maskv = cp.tile([128, 1], U32)
            nc.vector.memset(maskv[:], 0xFFFFF000)

            # ================= head: B (xn = xa * bcast(rrow)), C, Q ========
            with tc.tile_pool(name="pmb", bufs=3, space="PSUM") as pmb:
                # B3: xn = xa * bcast(rrow)
                for c in range(N // 512):
                    pb = pmb.tile([C, 512], F32, space="PSUM", tag="pb")
                    nc.tensor.matmul(out=pb[:], lhsT=one64_t[:],
                                     rhs=rrow[:, c * 512:(c + 1) * 512],
                                     start=True, stop=True)
                    nc.vector.tensor_tensor(out=xn[:, c * 512:(c + 1) * 512],
                                            in0=xa_t[0:C, c * 512:(c + 1) * 512].bitcast(F32),
                                            in1=pb[:], op=ALU.mult)

            # C: k|v table + Q (separate PSUM scope)
            with tc.tile_pool(name="pmc", bufs=3, space="PSUM") as pmh,\
                 tc.tile_pool(name="pmq2", bufs=2, space="PSUM") as pmq2:
                def emit_c(t):
                    pkv = pmh.tile([128, 2 * OC], F32, space="PSUM", tag="pkv")
                    nc.tensor.matmul(out=pkv[:], lhsT=xa_t[:, t * 128:(t + 1) * 128],
                                     rhs=wkvq_t[:, 0:2 * OC], start=True, stop=True)
                    ksq = wp_.tile([128, OC], F32, tag="ksq")
                    nc.scalar.activation(out=ksq[:], in_=pkv[:, 0:OC], func=AF.Square)
                    rkn = wp_.tile([128, NH], F32, tag="rkn")
                    nc.vector.tensor_reduce(
                        out=rkn[:], in_=ksq[:].rearrange("p (h f) -> p h f", h=NH),
                        axis=AXX, op=ALU.add)
                    nc.scalar.activation(out=rkn[:], in_=rkn[:],
                                         func=AF.Abs_reciprocal_sqrt)
                    tab = tabp.tile([128, 2 * OC], F16, tag="tab")
                    nc.vector.tensor_tensor(
                        out=tab[:, 0:OC].rearrange("p (h f) -> p h f", h=NH),
                        in0=pkv[:, 0:OC].rearrange("p (h f) -> p h f", h=NH),
                        in1=rkn[:].rearrange("p h -> p h ()").to_broadcast([128, NH, 64]),
                        op=ALU.mult)
                    nc.scalar.activation(out=tab[:, OC:2 * OC], in_=pkv[:, OC:2 * OC],
                                         func=AF.Copy)
                    nc.sync.dma_start(out=ktable[t * 128:(t + 1) * 128, :], in_=tab[:])

                # Q: q = Wq xr (+bq), node-major fp16, normalized, transposed
                def emit_qh(t):
                    pq = pmh.tile([128, OC], F32, space="PSUM", tag="pq")
                    nc.tensor.matmul(out=pq[:], lhsT=xr_t[:, t * 128:(t + 1) * 128],
                                     rhs=wkvq_t[:, 2 * OC:3 * OC], start=True, stop=True)
                    qsq = wp_.tile([128, OC], F32, tag="qsq")
                    nc.scalar.activation(out=qsq[:], in_=pq[:], func=AF.Square)
                    rq = wp_.tile([128, NH], F32, tag="rq")
                    nc.vector.tensor_reduce(
                        out=rq[:], in_=qsq[:].rearrange("p (h f) -> p h f", h=NH),
                        axis=AXX, op=ALU.add)
                    nc.scalar.activation(out=rq[:], in_=rq[:],
                                         func=AF.Abs_reciprocal_sqrt)
                    nc.vector.tensor_tensor(
                        out=qn_own[:, t, :].rearrange("p (h f) -> p h f", h=NH),
                        in0=pq[:].rearrange("p (h f) -> p h f", h=NH),
                        in1=rq[:].rearrange("p h -> p h ()").to_broadcast([128, NH, 64]),
                        op=ALU.mult)
                    for b_ in range(2):
                        qt = pmq2.tile([128, 128], F16, space="PSUM", tag="qt")
                        nc.tensor.transpose(out=qt[:], in_=qn_own[:, t, b_ * 128:(b_ + 1) * 128],
                                            identity=i128h_t[:])
                        nc.scalar.activation(out=(qc0 if b_ == 0 else qc1)[:, t * 128:(t + 1) * 128],
                                             in_=qt[:], func=AF.Copy)
                for t in range(N // 128):
                    emit_c(t)
                    if t % 2 == 1:
                        emit_qh(t // 2)

            # ================= D/E: sim -> top5 -> gather -> laplacian product
            with tc.tile_pool(name="pms", bufs=2, space="PSUM") as pms:

                def emit_d(t):
                    v8 = wp_.tile([128, 16], F32, tag="v8")
                    i8 = wp_.tile([128, 16], U32, tag="i8")
                    packed = wp_.tile([128, 16], U32, tag="pk")
                    for half in range(2):
                        sim = pms.tile([128, 2048], F32, space="PSUM", tag="sim")
                        for c in range(4):
                            cc = half * 4 + c
                            nc.tensor.matmul(out=sim[:, c * 512:(c + 1) * 512],
                                             lhsT=xr_t[0:C, t * 128:(t + 1) * 128],
                                             rhs=xn[:, cc * 512:(cc + 1) * 512],
                                             start=True, stop=True)
                        sl = slice(half * 8, half * 8 + 8)
                        nc.vector.max(out=v8[:, sl], in_=sim[:])
                        nc.vector.max_index(out=i8[:, sl], in_max=v8[:, sl],
                                            in_values=sim[:])
                        if half == 1:
                            nc.vector.tensor_scalar(out=i8[:, sl], in0=i8[:, sl],
                                                    scalar1=0x800, scalar2=None,
                                                    op0=ALU.bitwise_or)
                        nc.vector.scalar_tensor_tensor(
                            out=packed[:, sl], in0=v8[:, sl].bitcast(U32),
                            scalar=maskv[:, 0:1], in1=i8[:, sl],
                            op0=ALU.bitwise_and, op1=ALU.bitwise_or)
                    gpk = wp_.tile([128, 8], F32, tag="gpk")
                    nc.vector.max(out=gpk[:], in_=packed[:].bitcast(F32))
                    nc.vector.tensor_scalar(out=idx_all[:, t * 8:t * 8 + 8],
                                            in0=gpk[:].bitcast(U32), scalar1=0xFFF,
                                            scalar2=None, op0=ALU.bitwise_and)
                    gbuf = gp.tile([128, K, 2 * OC], F16, tag="gbuf")
                    for g in range(K):
                        nc.gpsimd.indirect_dma_start(
                            out=gbuf[:, g, :], out_offset=None, in_=ktable[:],
                            in_offset=bass.IndirectOffsetOnAxis(
                                ap=idx_all[:, t * 8 + g:t * 8 + g + 1], axis=0),
                        )
                    return gbuf

                def emit_e(t, gbuf):
                    # S = 2*g0 + g1 + g2 + g3 + g4 (self-loop: g0 is always self)
                    s1 = ep.tile([128, 2 * OC], F16, tag="s1")
                    nc.vector.scalar_tensor_tensor(
                        out=s1[:], in0=gbuf[:, 0, :], scalar=2.0, in1=gbuf[:, 1, :],
                        op0=ALU.mult, op1=ALU.add)
                    s2 = ep.tile([128, 2 * OC], F16, tag="s2")
                    nc.gpsimd.tensor_tensor(out=s2[:], in0=gbuf[:, 2, :],
                                            in1=gbuf[:, 3, :], op=ALU.add)
                    nc.gpsimd.tensor_tensor(out=s2[:], in0=s2[:],
                                            in1=gbuf[:, 4, :], op=ALU.add)
                    nc.vector.scalar_tensor_tensor(
                        out=s1[:], in0=s2[:], scalar=1.0, in1=s1[:],
                        op0=ALU.mult, op1=ALU.add)
                    prod = ep.tile([128, OC], F16, tag="prod")
                    nc.vector.scalar_tensor_tensor(
                        out=prod[:], in0=s1[:, 0:OC], scalar=1.0, in1=s1[:, OC:2 * OC],
                        op0=ALU.mult, op1=ALU.mult)
                    if t == 0:
                        nc.vector.tensor_copy(out=acc[:], in_=prod[:])
                    else:
                        nc.vector.tensor_tensor(out=acc[:], in0=acc[:], in1=prod[:],
                                                op=ALU.add)

                gbufs = {}
                for t in range(NT):
                    gbufs[t] = emit_d(t)
                    if t >= 3:
                        emit_e(t - 3, gbufs.pop(t - 3))
                for t in range(NT - 3, NT):
                    emit_e(t, gbufs.pop(t))

            # ================= F: kv reduce + AllReduce + fold; H: conv chain
            with tc.tile_pool(name="pmz", bufs=3, space="PSUM") as pmz:
                pkvs = pmz.tile([1, OC], F32, space="PSUM", tag="kv")
                nc.tensor.matmul(out=pkvs[:], lhsT=c36_t[:], rhs=acc[:],
                                 start=True, stop=True)
                kvs = wp_.tile([1, OC], F32, tag="kvs")
                nc.scalar.activation(out=kvs[:], in_=pkvs[:], func=AF.Copy)
                kv_in = dp.tile([1, OC], F32)
                kv_out = dp.tile([1, OC], F32)
                nc.sync.dma_start(out=kv_in[:], in_=kvs[:])
                if collective:
                    nc.gpsimd.collective_compute(
                        "AllReduce", ALU.add,
                        replica_groups=[[0, 1], [2, 3], [4, 5], [6, 7]],
                        ins=[kv_in[:].opt()], outs=[kv_out[:].opt()],
                    )
                else:
                    nc.sync.dma_start(out=kv_out[:], in_=kv_in[:])
                kvr = cp.tile([128, 2], F32)
                nc.sync.dma_start(out=kvr[:],
                                  in_=kv_out[:].rearrange("o (m p) -> o p m", m=2))

                wpk = cp.tile([128, 2, 64], F16)
                for m in range(2):
                    nc.vector.tensor_scalar_mul(out=wpk[:, m, :],
                                                in0=wpt_t[:, m, :],
                                                scalar1=kvr[:, m:m + 1])

                qcs = [qc0, qc1]
                for c in range(ROWS // 512):
                    cs = slice(c * 512, (c + 1) * 512)
                    pp1 = pmz.tile([64, 512], F32, space="PSUM", tag="pp")
                    for m in range(2):
                        nc.tensor.matmul(out=pp1[:], lhsT=wpk[:, m, :],
                                         rhs=qcs[m][:, cs],
                                         start=(m == 0), stop=(m == 1))
                    p1s = wp_.tile([64, 512], F16, tag="p1s")
                    nc.scalar.activation(out=p1s[:], in_=pp1[:], func=AF.Copy)
                    pp2 = pmz.tile([64, 512], F32, space="PSUM", tag="pp")
                    nc.tensor.matmul(out=pp2[:], lhsT=w1t_t[:], rhs=p1s[:],
                                     start=True, stop=True)
                    p2s = wp_.tile([64, 512], F16, tag="p2s")
                    nc.scalar.activation(out=p2s[:], in_=pp2[:], func=AF.Relu,
                                         bias=b1ff_t[:, 0:1])
                    pp3 = pmz.tile([64, 512], F32, space="PSUM", tag="pp")
                    nc.tensor.matmul(out=pp3[:], lhsT=w2t_t[:], rhs=p2s[:],
                                     start=True, stop=True)
                    outs = wp_.tile([64, 512], F32, tag="outs")
                    nc.scalar.activation(out=outs[:], in_=pp3[:], func=AF.Relu,
                                         bias=b2f_t[:, 0:1])
                    nc.sync.dma_start(out=out_half[:, cs], in_=outs[:])

    nc.compile()
    return nc


def _prep_inputs(inputs):
    f = lambda k: np.asarray(inputs[k], dtype=np.float32)
    x = f('x')
    wk, bk = f('wk'), f('bk')
    wq_, bq = f('wq'), f('bq')
    wv, bv = f('wv'), f('bv')
    wp, bp = f('wp'), f('bp')
    w1, b1 = f('w1'), f('b1')
    w2, b2 = f('w2'), f('b2')
    g1, beta1, m1, v1 = f('g1'), f('beta1'), f('m1'), f('v1')
    g2, beta2, m2, v2 = f('g2'), f('beta2'), f('m2'), f('v2')

    s1 = g1 / np.sqrt(v1 + BN_EPS)
    w1f = s1[:, None] * w1
    b1f = s1 * (b1 - m1) + beta1
    s2 = g2 / np.sqrt(v2 + BN_EPS)
    w2f = s2[:, None] * w2
    b2f_v = s2 * (b2 - m2) + beta2
    b1ff = w1f @ bp + b1f  # bp folded through w1f

    wkvq = np.zeros((C + 1, 3 * OC), np.float32)
    wkvq[0:C, 0:OC] = wk.T
    wkvq[C, 0:OC] = bk
    wkvq[0:C, OC:2 * OC] = wv.T
    wkvq[C, OC:2 * OC] = bv
    wkvq[0:C, 2 * OC:] = wq_.T
    wkvq[C, 2 * OC:] = bq
    wpt = np.ascontiguousarray(wp.T.reshape(2, 128, 64).transpose(1, 0, 2))

    shared = {
        "wkvq": wkvq, "wpt": wpt,
        "w1t": np.ascontiguousarray(w1f.T).astype(np.float16),
        "w2t": np.ascontiguousarray(w2f.T).astype(np.float16),
        "b1ff": b1ff.reshape(64, 1), "b2f": b2f_v.reshape(64, 1),
        "one64": np.ones((1, 64), np.float32),
        "i128h": np.eye(128, dtype=np.float16),
        "c36": np.full((128, 1), 1.0 / 36.0, np.float32),
    }
    in_maps = []
    for core in range(8):
        b = core // 2
        roff = (core % 2) * ROWS
        xa = np.ones((C + 1, N), np.float32)
        xa[0:C] = x[b].reshape(C, N)
        m = dict(shared)
        m["xa"] = xa
        m["xr"] = np.ascontiguousarray(xa[:, roff:roff + ROWS])
        xb = x[b].reshape(C, N)
        m["rrow_in"] = (1.0 / np.sqrt((xb * xb).sum(0))).reshape(1, N)
        in_maps.append(m)
    return in_maps


def kernel(**inputs):
    if "nc" not in _CACHE:
        _CACHE["nc"] = _build_bass()
    nc = _CACHE["nc"]
    in_maps = _prep_inputs(inputs)
    res = run_bass_kernel_spmd(nc, in_maps, list(range(8)))
    out = np.empty((B, 64, N), np.float32)
    for core in range(8):
        b = core // 2
        roff = (core % 2) * ROWS
        out[b][:, roff:roff + ROWS] = res.results[core]["out_half"]
    return out.reshape(B, 64, H, W)


if __name__ == "__main__":
    import os
    os.environ.setdefault("JAX_PLATFORMS", "cpu")
    import reference as R
    inputs = R.setup_inputs()
    expected = np.asarray(R.reference(**inputs))
    actual = kernel(**{k: np.asarray(v) for k, v in inputs.items()})
    rel = np.linalg.norm(actual - expected) / np.linalg.norm(expected)
    print("Relative error:", rel)


# revision 22
# speedup vs baseline: 1.1087x; 1.1087x over previous
"""HSpatialHyperGCN Trainium2 kernel (v2).

Shapes (hardcoded): x (4, 64, 64, 64); N = 4096 nodes per batch; 4 heads x 64
inter channels; top-5 cosine-similarity hypergraph; uniform degree 6 Laplacian;
hydra attention (global kv); 1x1-conv + folded-BN chain.

Sharding: 8 cores = 4 batches x 2 node-halves (core = 2*b + half).

v2 design vs v1 baseline (627 us):
  - all big matmuls in float32r (1 cycle/row vs 4 for fp32, ~1.5e-4 rel err)
  - sim rows scanned with MAX8/FIND_INDEX8 *directly on PSUM* in two
    2048-column halves (no PSUM->SBUF copy of the 8.4M-entry sim matrix)
  - halves merged with a bit-pack trick: (value & 0xFFFFF000) | column_index,
    max8 over the 16 packed candidates, AND-decode -> exact-ish top-5 with
    index tie-break, no duplicate-index pathology
  - pack/decode on gpsimd (feeds its own indirect gathers), scan on DVE,
    table normalization on scalar, conv chains on PE: engines balanced so the
    irreducible DVE scan (~140 us) is the critical path
  - k*v Laplacian product accumulated on DVE in fp16; kv reduced via a
    (1/36)-weighted ones-matmul; AllReduce over batch pairs; kv folded into wp
  - lhsT of sim = raw x rows (row scale cannot change a row's top-k)
"""

import sys

sys.path.insert(0, "/opt/trn_rl_repo")

import numpy as np

from concourse import bass, mybir, tile, bacc
from concourse.bass_utils import run_bass_kernel_spmd

F32 = mybir.dt.float32
F32R = mybir.dt.float32r
F16 = mybir.dt.float16
U32 = mybir.dt.uint32
AF = mybir.ActivationFunctionType
ALU = mybir.AluOpType
AXX = mybir.AxisListType.X

B, C, H, W = 4, 64, 64, 64
N = H * W            # 4096
NH = 4
INTER = 64
OC = NH * INTER      # 256
K = 5
ROWS = N // 2        # 2048 rows per core
NT = ROWS // 128     # 16 row tiles per core
BN_EPS = 1e-5

_CACHE = {}


def _build_bass(collective=True):
    nc = bacc.Bacc(None, target_bir_lowering=False, debug=False, num_devices=8)

    xa = nc.dram_tensor("xa", [C + 1, N], F32R, kind="ExternalInput")
    xr = nc.dram_tensor("xr", [C + 1, ROWS], F32R, kind="ExternalInput")
    wkvq = nc.dram_tensor("wkvq", [C + 1, 3 * OC], F32R, kind="ExternalInput")
    wpt = nc.dram_tensor("wpt", [128, 2, 64], F32, kind="ExternalInput")
    w1t = nc.dram_tensor("w1t", [64, 64], F16, kind="ExternalInput")
    w2t = nc.dram_tensor("w2t", [64, 64], F16, kind="ExternalInput")
    b1ff = nc.dram_tensor("b1ff", [64, 1], F32, kind="ExternalInput")
    b2f = nc.dram_tensor("b2f", [64, 1], F32, kind="ExternalInput")
    one64 = nc.dram_tensor("one64", [1, 64], F32R, kind="ExternalInput")
    i128h = nc.dram_tensor("i128h", [128, 128], F16, kind="ExternalInput")
    rrow_in = nc.dram_tensor("rrow_in", [1, N], F32R, kind="ExternalInput")
    c36 = nc.dram_tensor("c36", [128, 1], F32, kind="ExternalInput")

    out_half = nc.dram_tensor("out_half", [64, ROWS], F32, kind="ExternalOutput")

    ktable = nc.dram_tensor("ktable", [N, 2 * OC], F16)  # internal per-core DRAM

    with tile.TileContext(nc) as tc:
        with (
            tc.tile_pool(name="const", bufs=1) as cp,
            tc.tile_pool(name="work", bufs=6) as wp_,
            tc.tile_pool(name="tabp", bufs=4) as tabp,
            tc.tile_pool(name="gp", bufs=4) as gp,
            tc.tile_pool(name="ep", bufs=2) as ep,
            tc.tile_pool(name="dram", bufs=2, space="DRAM") as dp,
        ):
            # ---- persistent loads
            xa_t = cp.tile([C + 1, N], F32R)
            for j in range(8):
                eng = nc.sync if j % 2 == 0 else nc.scalar
                eng.dma_start(out=xa_t[:, j * 512:(j + 1) * 512],
                              in_=xa[:, j * 512:(j + 1) * 512])
            xr_t = cp.tile([C + 1, ROWS], F32R)
            for j in range(4):
                eng = nc.sync if j % 2 == 0 else nc.scalar
                eng.dma_start(out=xr_t[:, j * 512:(j + 1) * 512],
                              in_=xr[:, j * 512:(j + 1) * 512])
            wkvq_t = cp.tile([C + 1, 3 * OC], F32R)
            for j in range(2):
                eng = nc.sync if j % 2 == 0 else nc.scalar
                eng.dma_start(out=wkvq_t[:, j * 384:(j + 1) * 384],
                              in_=wkvq[:, j * 384:(j + 1) * 384])
            wpt_t = cp.tile([128, 2, 64], F32)
            nc.sync.dma_start(out=wpt_t[:], in_=wpt[:])
            w1t_t = cp.tile([64, 64], F16)
            nc.sync.dma_start(out=w1t_t[:], in_=w1t[:])
            w2t_t = cp.tile([64, 64], F16)
            nc.sync.dma_start(out=w2t_t[:], in_=w2t[:])
            b1ff_t = cp.tile([64, 1], F32)
            nc.sync.dma_start(out=b1ff_t[:], in_=b1ff[:])
            b2f_t = cp.tile([64, 1], F32)
            nc.sync.dma_start(out=b2f_t[:], in_=b2f[:])
            one64_t = cp.tile([1, 64], F32R)
            nc.sync.dma_start(out=one64_t[:], in_=one64[:])
            rrow = cp.tile([1, N], F32R)
            nc.scalar.dma_start(out=rrow[:], in_=rrow_in[:])
            c36_t = cp.tile([128, 1], F32)
            nc.sync.dma_start(out=c36_t[:], in_=c36[:])
            i128h_t = cp.tile([128, 128], F16)
            nc.sync.dma_start(out=i128h_t[:], in_=i128h[:])

            xn = cp.tile([C, N], F32R)
            qn_own = cp.tile([128, NT, OC], F16)
            qc0 = cp.tile([128, ROWS], F16)
            qc1 = cp.tile([128, ROWS], F16)
            idx_all = cp.tile([128, NT * 8], U32)
            acc = cp.tile([128, OC], F32)
            maskv = cp.tile([128, 1], U32)
            nc.vector.memset(maskv[:], 0xFFFFF000)

            # ================= head: B (xn = xa * bcast(rrow)), C, Q ========
            with tc.tile_pool(name="pmb", bufs=3, space="PSUM") as pmb:
                # B3: xn = xa * bcast(rrow)
                for c in range(N // 512):
                    pb = pmb.tile([C, 512], F32, space="PSUM", tag="pb")
                    nc.tensor.matmul(out=pb[:], lhsT=one64_t[:],
                                     rhs=rrow[:, c * 512:(c + 1) * 512],
                                     start=True, stop=True)
                    nc.vector.tensor_tensor(out=xn[:, c * 512:(c + 1) * 512],
                                            in0=xa_t[0:C, c * 512:(c + 1) * 512].bitcast(F32),
                                            in1=pb[:], op=ALU.mult)

            # C: k|v table + Q (separate PSUM scope)
            with tc.tile_pool(name="pmc", bufs=3, space="PSUM") as pmh,\
                 tc.tile_pool(name="pmq2", bufs=2, space="PSUM") as pmq2:
                def emit_c(t):
                    pkv = pmh.tile([128, 2 * OC], F32, space="PSUM", tag="pkv")
                    nc.tensor.matmul(out=pkv[:], lhsT=xa_t[:, t * 128:(t + 1) * 128],
                                     rhs=wkvq_t[:, 0:2 * OC], start=True, stop=True)
                    ksq = wp_.tile([128, OC], F32, tag="ksq")
                    nc.scalar.activation(out=ksq[:], in_=pkv[:, 0:OC], func=AF.Square)
                    rkn = wp_.tile([128, NH], F32, tag="rkn")
                    nc.vector.tensor_reduce(
                        out=rkn[:], in_=ksq[:].rearrange("p (h f) -> p h f", h=NH),
                        axis=AXX, op=ALU.add)
                    nc.scalar.activation(out=rkn[:], in_=rkn[:],
                                         func=AF.Abs_reciprocal_sqrt)
                    tab = tabp.tile([128, 2 * OC], F16, tag="tab")
                    nc.vector.tensor_tensor(
                        out=tab[:, 0:OC].rearrange("p (h f) -> p h f", h=NH),
                        in0=pkv[:, 0:OC].rearrange("p (h f) -> p h f", h=NH),
                        in1=rkn[:].rearrange("p h -> p h ()").to_broadcast([128, NH, 64]),
                        op=ALU.mult)
                    nc.scalar.activation(out=tab[:, OC:2 * OC], in_=pkv[:, OC:2 * OC],
                                         func=AF.Copy)
                    nc.sync.dma_start(out=ktable[t * 128:(t + 1) * 128, :], in_=tab[:])

                # Q: q = Wq xr (+bq), node-major fp16, normalized, transposed
                def emit_qh(t):
                    pq = pmh.tile([128, OC], F32, space="PSUM", tag="pq")
                    nc.tensor.matmul(out=pq[:], lhsT=xr_t[:, t * 128:(t + 1) * 128],
                                     rhs=wkvq_t[:, 2 * OC:3 * OC], start=True, stop=True)
                    qsq = wp_.tile([128, OC], F32, tag="qsq")
                    nc.scalar.activation(out=qsq[:], in_=pq[:], func=AF.Square)
                    rq = wp_.tile([128, NH], F32, tag="rq")
                    nc.vector.tensor_reduce(
                        out=rq[:], in_=qsq[:].rearrange("p (h f) -> p h f", h=NH),
                        axis=AXX, op=ALU.add)
                    nc.scalar.activation(out=rq[:], in_=rq[:],
                                         func=AF.Abs_reciprocal_sqrt)
                    nc.vector.tensor_tensor(
                        out=qn_own[:, t, :].rearrange("p (h f) -> p h f", h=NH),
                        in0=pq[:].rearrange("p (h f) -> p h f", h=NH),
                        in1=rq[:].rearrange("p h -> p h ()").to_broadcast([128, NH, 64]),
                        op=ALU.mult)
                    for b_ in range(2):
                        qt = pmq2.tile([128, 128], F16, space="PSUM", tag="qt")
                        nc.tensor.transpose(out=qt[:], in_=qn_own[:, t, b_ * 128:(b_ + 1) * 128],
                                            identity=i128h_t[:])
                        nc.scalar.activation(out=(qc0 if b_ == 0 else qc1)[:, t * 128:(t + 1) * 128],
                                             in_=qt[:], func=AF.Copy)
                for t in range(N // 128):
                    emit_c(t)
                    if t % 2 == 1:
                        emit_qh(t // 2)

            # ================= D/E: sim -> top5 -> gather -> laplacian product
            with tc.tile_pool(name="pms", bufs=2, space="PSUM") as pms:

                def emit_d(t):
                    v8 = wp_.tile([128, 16], F32, tag="v8")
                    i8 = wp_.tile([128, 16], U32, tag="i8")
                    packed = wp_.tile([128, 16], U32, tag="pk")
                    for half in range(2):
                        sim = pms.tile([128, 2048], F32, space="PSUM", tag="sim")
                        for c in range(4):
                            cc = half * 4 + c
                            nc.tensor.matmul(out=sim[:, c * 512:(c + 1) * 512],
                                             lhsT=xr_t[0:C, t * 128:(t + 1) * 128],
                                             rhs=xn[:, cc * 512:(cc + 1) * 512],
                                             start=True, stop=True)
                        sl = slice(half * 8, half * 8 + 8)
                        nc.vector.max(out=v8[:, sl], in_=sim[:])
                        nc.vector.max_index(out=i8[:, sl], in_max=v8[:, sl],
                                            in_values=sim[:])
                    nc.vector.tensor_scalar(out=i8[:, 8:16], in0=i8[:, 8:16],
                                            scalar1=0x800, scalar2=None,
                                            op0=ALU.bitwise_or)
                    nc.vector.scalar_tensor_tensor(
                        out=packed[:], in0=v8[:].bitcast(U32),
                        scalar=maskv[:, 0:1], in1=i8[:],
                        op0=ALU.bitwise_and, op1=ALU.bitwise_or)
                    gpk = wp_.tile([128, 8], F32, tag="gpk")
                    nc.vector.max(out=gpk[:], in_=packed[:].bitcast(F32))
                    nc.vector.tensor_scalar(out=idx_all[:, t * 8:t * 8 + 8],
                                            in0=gpk[:].bitcast(U32), scalar1=0xFFF,
                                            scalar2=None, op0=ALU.bitwise_and)
                    gbuf = gp.tile([128, K, 2 * OC], F16, tag="gbuf")
                    for g in range(K):
                        nc.gpsimd.indirect_dma_start(
                            out=gbuf[:, g, :], out_offset=None, in_=ktable[:],
                            in_offset=bass.IndirectOffsetOnAxis(
                                ap=idx_all[:, t * 8 + g:t * 8 + g + 1], axis=0),
                        )
                    return gbuf

                def emit_e(t, gbuf):
                    # S = 2*g0 + g1 + g2 + g3 + g4 (self-loop: g0 is always self)
                    s1 = ep.tile([128, 2 * OC], F16, tag="s1")
                    nc.vector.scalar_tensor_tensor(
                        out=s1[:], in0=gbuf[:, 0, :], scalar=2.0, in1=gbuf[:, 1, :],
                        op0=ALU.mult, op1=ALU.add)
                    s2 = ep.tile([128, 2 * OC], F16, tag="s2")
                    nc.gpsimd.tensor_tensor(out=s2[:], in0=gbuf[:, 2, :],
                                            in1=gbuf[:, 3, :], op=ALU.add)
                    nc.gpsimd.tensor_tensor(out=s2[:], in0=s2[:],
                                            in1=gbuf[:, 4, :], op=ALU.add)
                    nc.vector.scalar_tensor_tensor(
                        out=s1[:], in0=s2[:], scalar=1.0, in1=s1[:],
                        op0=ALU.mult, op1=ALU.add)
                    prod = ep.tile([128, OC], F16, tag="prod")
                    nc.gpsimd.tensor_tensor(out=prod[:], in0=s1[:, 0:OC],
                                            in1=s1[:, OC:2 * OC], op=ALU.mult)
                    if t == 0:
                        nc.vector.tensor_copy(out=acc[:], in_=prod[:])
                    else:
                        nc.vector.tensor_tensor(out=acc[:], in0=acc[:], in1=prod[:],
                                                op=ALU.add)

                gbufs = {}
                for t in range(NT):
                    gbufs[t] = emit_d(t)
                    if t >= 3:
                        emit_e(t - 3, gbufs.pop(t - 3))
                for t in range(NT - 3, NT):
                    emit_e(t, gbufs.pop(t))

            # ================= F: kv reduce + AllReduce + fold; H: conv chain
            with tc.tile_pool(name="pmz", bufs=3, space="PSUM") as pmz:
                pkvs = pmz.tile([1, OC], F32, space="PSUM", tag="kv")
                nc.tensor.matmul(out=pkvs[:], lhsT=c36_t[:], rhs=acc[:],
                                 start=True, stop=True)
                kvs = wp_.tile([1, OC], F32, tag="kvs")
                nc.scalar.activation(out=kvs[:], in_=pkvs[:], func=AF.Copy)
                kv_in = dp.tile([1, OC], F32)
                kv_out = dp.tile([1, OC], F32)
                nc.sync.dma_start(out=kv_in[:], in_=kvs[:])
                if collective:
                    nc.gpsimd.collective_compute(
                        "AllReduce", ALU.add,
                        replica_groups=[[0, 1], [2, 3], [4, 5], [6, 7]],
                        ins=[kv_in[:].opt()], outs=[kv_out[:].opt()],
                    )
                else:
                    nc.sync.dma_start(out=kv_out[:], in_=kv_in[:])
                kvr = cp.tile([128, 2], F32)
                nc.sync.dma_start(out=kvr[:],
                                  in_=kv_out[:].rearrange("o (m p) -> o p m", m=2))

                wpk = cp.tile([128, 2, 64], F16)
                for m in range(2):
                    nc.vector.tensor_scalar_mul(out=wpk[:, m, :],
                                                in0=wpt_t[:, m, :],
                                                scalar1=kvr[:, m:m + 1])

                qcs = [qc0, qc1]
                for c in range(ROWS // 512):
                    cs = slice(c * 512, (c + 1) * 512)
                    pp1 = pmz.tile([64, 512], F32, space="PSUM", tag="pp")
                    for m in range(2):
                        nc.tensor.matmul(out=pp1[:], lhsT=wpk[:, m, :],
                                         rhs=qcs[m][:, cs],
                                         start=(m == 0), stop=(m == 1))
                    p1s = wp_.tile([64, 512], F16, tag="p1s")
                    nc.scalar.activation(out=p1s[:], in_=pp1[:], func=AF.Copy)
                    pp2 = pmz.tile([64, 512], F32, space="PSUM", tag="pp")
                    nc.tensor.matmul(out=pp2[:], lhsT=w1t_t[:], rhs=p1s[:],
                                     start=True, stop=True)
                    p2s = wp_.tile([64, 512], F16, tag="p2s")
                    nc.scalar.activation(out=p2s[:], in_=pp2[:], func=AF.Relu,
                                         bias=b1ff_t[:, 0:1])
                    pp3 = pmz.tile([64, 512], F32, space="PSUM", tag="pp")
                    nc.tensor.matmul(out=pp3[:], lhsT=w2t_t[:], rhs=p2s[:],
                                     start=True, stop=True)
                    outs = wp_.tile([64, 512], F32, tag="outs")
                    nc.scalar.activation(out=outs[:], in_=pp3[:], func=AF.Relu,
                                         bias=b2f_t[:, 0:1])
                    (nc.sync if c % 2 == 0 else nc.scalar).dma_start(
                        out=out_half[:, cs], in_=outs[:])

    nc.compile()
    return nc


def _prep_inputs(inputs):
    f = lambda k: np.asarray(inputs[k], dtype=np.float32)
    x = f('x')
    wk, bk = f('wk'), f('bk')
    wq_, bq = f('wq'), f('bq')
    wv, bv = f('wv'), f('bv')
    wp, bp = f('wp'), f('bp')
    w1, b1 = f('w1'), f('b1')
    w2, b2 = f('w2'), f('b2')
    g1, beta1, m1, v1 = f('g1'), f('beta1'), f('m1'), f('v1')
    g2, beta2, m2, v2 = f('g2'), f('beta2'), f('m2'), f('v2')

    s1 = g1 / np.sqrt(v1 + BN_EPS)
    w1f = s1[:, None] * w1
    b1f = s1 * (b1 - m1) + beta1
    s2 = g2 / np.sqrt(v2 + BN_EPS)
    w2f = s2[:, None] * w2
    b2f_v = s2 * (b2 - m2) + beta2
    b1ff = w1f @ bp + b1f  # bp folded through w1f

    wkvq = np.zeros((C + 1, 3 * OC), np.float32)
    wkvq[0:C, 0:OC] = wk.T
    wkvq[C, 0:OC] = bk
    wkvq[0:C, OC:2 * OC] = wv.T
    wkvq[C, OC:2 * OC] = bv
    wkvq[0:C, 2 * OC:] = wq_.T
    wkvq[C, 2 * OC:] = bq
    wpt = np.ascontiguousarray(wp.T.reshape(2, 128, 64).transpose(1, 0, 2))

    shared = {
        "wkvq": wkvq, "wpt": wpt,
        "w1t": np.ascontiguousarray(w1f.T).astype(np.float16),
        "w2t": np.ascontiguousarray(w2f.T).astype(np.float16),
        "b1ff": b1ff.reshape(64, 1), "b2f": b2f_v.reshape(64, 1),
        "one64": np.ones((1, 64), np.float32),
        "i128h": np.eye(128, dtype=np.float16),
        "c36": np.full((128, 1), 1.0 / 36.0, np.float32),
    }
    in_maps = []
    for core in range(8):
        b = core // 2
        roff = (core % 2) * ROWS
        xa = np.ones((C + 1, N), np.float32)
        xa[0:C] = x[b].reshape(C, N)
        m = dict(shared)
        m["xa"] = xa
        m["xr"] = np.ascontiguousarray(xa[:, roff:roff + ROWS])
        xb = x[b].reshape(C, N)
        m["rrow_in"] = (1.0 / np.sqrt((xb * xb).sum(0))).reshape(1, N)
        in_maps.append(m)
    return in_maps


def kernel(**inputs):
    if "nc" not in _CACHE:
        _CACHE["nc"] = _build_bass()
    nc = _CACHE["nc"]
    in_maps = _prep_inputs(inputs)
    res = run_bass_kernel_spmd(nc, in_maps, list(range(8)))
    out = np.empty((B, 64, N), np.float32)
    for core in range(8):
        b = core // 2
        roff = (core % 2) * ROWS
        out[b][:, roff:roff + ROWS] = res.results[core]["out_half"]
    return out.reshape(B, 64, H, W)


if __name__ == "__main__":
    import os
    os.environ.setdefault("JAX_PLATFORMS", "cpu")
    import reference as R
    inputs = R.setup_inputs()
    expected = np.asarray(R.reference(**inputs))
    actual = kernel(**{k: np.asarray(v) for k, v in inputs.items()})
    rel = np.linalg.norm(actual - expected) / np.linalg.norm(expected)
    print("Relative error:", rel)


# revision 24
# speedup vs baseline: 1.1088x; 1.0001x over previous
"""HSpatialHyperGCN Trainium2 kernel (v2).

Shapes (hardcoded): x (4, 64, 64, 64); N = 4096 nodes per batch; 4 heads x 64
inter channels; top-5 cosine-similarity hypergraph; uniform degree 6 Laplacian;
hydra attention (global kv); 1x1-conv + folded-BN chain.

Sharding: 8 cores = 4 batches x 2 node-halves (core = 2*b + half).

v2 design vs v1 baseline (627 us):
  - all big matmuls in float32r (1 cycle/row vs 4 for fp32, ~1.5e-4 rel err)
  - sim rows scanned with MAX8/FIND_INDEX8 *directly on PSUM* in two
    2048-column halves (no PSUM->SBUF copy of the 8.4M-entry sim matrix)
  - halves merged with a bit-pack trick: (value & 0xFFFFF000) | column_index,
    max8 over the 16 packed candidates, AND-decode -> exact-ish top-5 with
    index tie-break, no duplicate-index pathology
  - pack/decode on gpsimd (feeds its own indirect gathers), scan on DVE,
    table normalization on scalar, conv chains on PE: engines balanced so the
    irreducible DVE scan (~140 us) is the critical path
  - k*v Laplacian product accumulated on DVE in fp16; kv reduced via a
    (1/36)-weighted ones-matmul; AllReduce over batch pairs; kv folded into wp
  - lhsT of sim = raw x rows (row scale cannot change a row's top-k)
"""

import sys

sys.path.insert(0, "/opt/trn_rl_repo")

import numpy as np

from concourse import bass, bass_isa, mybir, tile, bacc
from concourse.bass_utils import run_bass_kernel_spmd

F32 = mybir.dt.float32
F32R = mybir.dt.float32r
F16 = mybir.dt.float16
U32 = mybir.dt.uint32
AF = mybir.ActivationFunctionType
ALU = mybir.AluOpType
AXX = mybir.AxisListType.X

B, C, H, W = 4, 64, 64, 64
N = H * W            # 4096
NH = 4
INTER = 64
OC = NH * INTER      # 256
K = 5
ROWS = N // 2        # 2048 rows per core
NT = ROWS // 128     # 16 row tiles per core
BN_EPS = 1e-5

_CACHE = {}


def _build_bass(collective=True):
    nc = bacc.Bacc(None, target_bir_lowering=False, debug=False, num_devices=8)

    xa = nc.dram_tensor("xa", [C + 1, N], F32R, kind="ExternalInput")
    xr = nc.dram_tensor("xr", [C + 1, ROWS], F32R, kind="ExternalInput")
    wkvq = nc.dram_tensor("wkvq", [C + 1, 3 * OC], F32R, kind="ExternalInput")
    wpt = nc.dram_tensor("wpt", [128, 2, 64], F32, kind="ExternalInput")
    w1t = nc.dram_tensor("w1t", [64, 64], F16, kind="ExternalInput")
    w2t = nc.dram_tensor("w2t", [64, 64], F16, kind="ExternalInput")
    b1ff = nc.dram_tensor("b1ff", [64, 1], F32, kind="ExternalInput")
    b2f = nc.dram_tensor("b2f", [64, 1], F32, kind="ExternalInput")
    one64 = nc.dram_tensor("one64", [1, 64], F32R, kind="ExternalInput")
    i128h = nc.dram_tensor("i128h", [128, 128], F16, kind="ExternalInput")
    rrow_in = nc.dram_tensor("rrow_in", [1, N], F32R, kind="ExternalInput")
    c36 = nc.dram_tensor("c36", [128, 1], F32, kind="ExternalInput")

    out_half = nc.dram_tensor("out_half", [64, ROWS], F32, kind="ExternalOutput")

    ktable = nc.dram_tensor("ktable", [N, 2 * OC], F16)  # internal per-core DRAM

    with tile.TileContext(nc) as tc:
        with (
            tc.tile_pool(name="const", bufs=1) as cp,
            tc.tile_pool(name="work", bufs=6) as wp_,
            tc.tile_pool(name="tabp", bufs=4) as tabp,
            tc.tile_pool(name="gp", bufs=4) as gp,
            tc.tile_pool(name="ep", bufs=2) as ep,
            tc.tile_pool(name="dram", bufs=2, space="DRAM") as dp,
        ):
            # ---- persistent loads
            xa_t = cp.tile([C + 1, N], F32R)
            for j in range(8):
                eng = nc.sync if j % 2 == 0 else nc.scalar
                eng.dma_start(out=xa_t[:, j * 512:(j + 1) * 512],
                              in_=xa[:, j * 512:(j + 1) * 512])
            xr_t = cp.tile([C + 1, ROWS], F32R)
            for j in range(4):
                eng = nc.sync if j % 2 == 0 else nc.scalar
                eng.dma_start(out=xr_t[:, j * 512:(j + 1) * 512],
                              in_=xr[:, j * 512:(j + 1) * 512])
            wkvq_t = cp.tile([C + 1, 3 * OC], F32R)
            for j in range(2):
                eng = nc.sync if j % 2 == 0 else nc.scalar
                eng.dma_start(out=wkvq_t[:, j * 384:(j + 1) * 384],
                              in_=wkvq[:, j * 384:(j + 1) * 384])
            wpt_t = cp.tile([128, 2, 64], F32)
            nc.sync.dma_start(out=wpt_t[:], in_=wpt[:])
            w1t_t = cp.tile([64, 64], F16)
            nc.sync.dma_start(out=w1t_t[:], in_=w1t[:])
            w2t_t = cp.tile([64, 64], F16)
            nc.sync.dma_start(out=w2t_t[:], in_=w2t[:])
            b1ff_t = cp.tile([64, 1], F32)
            nc.sync.dma_start(out=b1ff_t[:], in_=b1ff[:])
            b2f_t = cp.tile([64, 1], F32)
            nc.sync.dma_start(out=b2f_t[:], in_=b2f[:])
            one64_t = cp.tile([1, 64], F32R)
            nc.sync.dma_start(out=one64_t[:], in_=one64[:])
            rrow = cp.tile([1, N], F32R)
            nc.scalar.dma_start(out=rrow[:], in_=rrow_in[:])
            c36_t = cp.tile([128, 1], F32)
            nc.sync.dma_start(out=c36_t[:], in_=c36[:])
            i128h_t = cp.tile([128, 128], F16)
            nc.sync.dma_start(out=i128h_t[:], in_=i128h[:])

            xn = cp.tile([C, N], F32R)
            qn_own = cp.tile([128, NT, OC], F16)
            qc0 = cp.tile([128, ROWS], F16)
            qc1 = cp.tile([128, ROWS], F16)
            idx_all = cp.tile([128, NT * 8], U32)
            acc = cp.tile([128, OC], F32)
            acc_b = cp.tile([128, OC], F32)
            maskv = cp.tile([128, 1], U32)
            nc.vector.memset(maskv[:], 0xFFFFF000)

            # ================= head: B (xn = xa * bcast(rrow)), C, Q ========
            with tc.tile_pool(name="pmb", bufs=3, space="PSUM") as pmb:
                # B3: xn = xa * bcast(rrow)
                for c in range(N // 512):
                    pb = pmb.tile([C, 512], F32, space="PSUM", tag="pb")
                    nc.tensor.matmul(out=pb[:], lhsT=one64_t[:],
                                     rhs=rrow[:, c * 512:(c + 1) * 512],
                                     start=True, stop=True)
                    nc.vector.tensor_tensor(out=xn[:, c * 512:(c + 1) * 512],
                                            in0=xa_t[0:C, c * 512:(c + 1) * 512].bitcast(F32),
                                            in1=pb[:], op=ALU.mult)

            # C: k|v table + Q (separate PSUM scope)
            with tc.tile_pool(name="pmc", bufs=3, space="PSUM") as pmh,\
                 tc.tile_pool(name="pmq2", bufs=2, space="PSUM") as pmq2:
                def emit_c(t):
                    pkv = pmh.tile([128, 2 * OC], F32, space="PSUM", tag="pkv")
                    nc.tensor.matmul(out=pkv[:], lhsT=xa_t[:, t * 128:(t + 1) * 128],
                                     rhs=wkvq_t[:, 0:2 * OC], start=True, stop=True)
                    ksq = wp_.tile([128, OC], F32, tag="ksq")
                    nc.scalar.activation(out=ksq[:], in_=pkv[:, 0:OC], func=AF.Square)
                    rkn = wp_.tile([128, NH], F32, tag="rkn")
                    nc.vector.tensor_reduce(
                        out=rkn[:], in_=ksq[:].rearrange("p (h f) -> p h f", h=NH),
                        axis=AXX, op=ALU.add)
                    nc.scalar.activation(out=rkn[:], in_=rkn[:],
                                         func=AF.Abs_reciprocal_sqrt)
                    tab = tabp.tile([128, 2 * OC], F16, tag="tab")
                    nc.vector.tensor_tensor(
                        out=tab[:, 0:OC].rearrange("p (h f) -> p h f", h=NH),
                        in0=pkv[:, 0:OC].rearrange("p (h f) -> p h f", h=NH),
                        in1=rkn[:].rearrange("p h -> p h ()").to_broadcast([128, NH, 64]),
                        op=ALU.mult)
                    nc.scalar.activation(out=tab[:, OC:2 * OC], in_=pkv[:, OC:2 * OC],
                                         func=AF.Copy)
                    nc.sync.dma_start(out=ktable[t * 128:(t + 1) * 128, :], in_=tab[:])

                # Q: q = Wq xr (+bq), node-major fp16, normalized, transposed
                def emit_qh(t):
                    pq = pmh.tile([128, OC], F32, space="PSUM", tag="pq")
                    nc.tensor.matmul(out=pq[:], lhsT=xr_t[:, t * 128:(t + 1) * 128],
                                     rhs=wkvq_t[:, 2 * OC:3 * OC], start=True, stop=True)
                    qsq = wp_.tile([128, OC], F32, tag="qsq")
                    nc.scalar.activation(out=qsq[:], in_=pq[:], func=AF.Square)
                    rq = wp_.tile([128, NH], F32, tag="rq")
                    nc.vector.tensor_reduce(
                        out=rq[:], in_=qsq[:].rearrange("p (h f) -> p h f", h=NH),
                        axis=AXX, op=ALU.add)
                    nc.scalar.activation(out=rq[:], in_=rq[:],
                                         func=AF.Abs_reciprocal_sqrt)
                    nc.vector.tensor_tensor(
                        out=qn_own[:, t, :].rearrange("p (h f) -> p h f", h=NH),
                        in0=pq[:].rearrange("p (h f) -> p h f", h=NH),
                        in1=rq[:].rearrange("p h -> p h ()").to_broadcast([128, NH, 64]),
                        op=ALU.mult)
                    for b_ in range(2):
                        qt = pmq2.tile([128, 128], F16, space="PSUM", tag="qt")
                        nc.tensor.transpose(out=qt[:], in_=qn_own[:, t, b_ * 128:(b_ + 1) * 128],
                                            identity=i128h_t[:])
                        nc.scalar.activation(out=(qc0 if b_ == 0 else qc1)[:, t * 128:(t + 1) * 128],
                                             in_=qt[:], func=AF.Copy)
                for t in range(N // 128):
                    emit_c(t)
                    if t % 2 == 1:
                        emit_qh(t // 2)

            # ================= D/E: sim -> top5 -> gather -> laplacian product
            with tc.tile_pool(name="pms", bufs=2, space="PSUM") as pms:

                def emit_d(t):
                    v8 = wp_.tile([128, 16], F32, tag="v8")
                    i8 = wp_.tile([128, 16], U32, tag="i8")
                    packed = wp_.tile([128, 16], U32, tag="pk")
                    for half in range(2):
                        sim = pms.tile([128, 2048], F32, space="PSUM", tag="sim")
                        for c in range(4):
                            cc = half * 4 + c
                            nc.tensor.matmul(out=sim[:, c * 512:(c + 1) * 512],
                                             lhsT=xr_t[0:C, t * 128:(t + 1) * 128],
                                             rhs=xn[:, cc * 512:(cc + 1) * 512],
                                             start=True, stop=True)
                        sl = slice(half * 8, half * 8 + 8)
                        nc.vector.max(out=v8[:, sl], in_=sim[:])
                        nc.vector.max_index(out=i8[:, sl], in_max=v8[:, sl],
                                            in_values=sim[:])
                    nc.vector.tensor_scalar(out=i8[:, 8:16], in0=i8[:, 8:16],
                                            scalar1=0x800, scalar2=None,
                                            op0=ALU.bitwise_or)
                    nc.vector.scalar_tensor_tensor(
                        out=packed[:], in0=v8[:].bitcast(U32),
                        scalar=maskv[:, 0:1], in1=i8[:],
                        op0=ALU.bitwise_and, op1=ALU.bitwise_or)
                    gpk = wp_.tile([128, 8], F32, tag="gpk")
                    nc.vector.max(out=gpk[:], in_=packed[:].bitcast(F32))
                    nc.vector.tensor_scalar(out=idx_all[:, t * 8:t * 8 + 8],
                                            in0=gpk[:].bitcast(U32), scalar1=0xFFF,
                                            scalar2=None, op0=ALU.bitwise_and)
                    gbuf = gp.tile([128, K, 2 * OC], F16, tag="gbuf")
                    for g in range(K):
                        nc.gpsimd.indirect_dma_start(
                            out=gbuf[:, g, :], out_offset=None, in_=ktable[:],
                            in_offset=bass.IndirectOffsetOnAxis(
                                ap=idx_all[:, t * 8 + g:t * 8 + g + 1], axis=0),
                        )
                    return gbuf

                def emit_e(t, gbuf):
                    # S = 2*g0 + g1 + g2 + g3 + g4 (self-loop: g0 is always self)
                    s1 = ep.tile([128, 2 * OC], F16, tag="s1")
                    nc.vector.scalar_tensor_tensor(
                        out=s1[:], in0=gbuf[:, 0, :], scalar=2.0, in1=gbuf[:, 1, :],
                        op0=ALU.mult, op1=ALU.add)
                    s2 = ep.tile([128, 2 * OC], F16, tag="s2")
                    nc.gpsimd.tensor_tensor(out=s2[:], in0=gbuf[:, 2, :],
                                            in1=gbuf[:, 3, :], op=ALU.add)
                    nc.gpsimd.tensor_tensor(out=s2[:], in0=s2[:],
                                            in1=gbuf[:, 4, :], op=ALU.add)
                    nc.vector.scalar_tensor_tensor(
                        out=s1[:], in0=s2[:], scalar=1.0, in1=s1[:],
                        op0=ALU.mult, op1=ALU.add)
                    prod = ep.tile([128, OC], F16, tag="prod")
                    nc.gpsimd.tensor_tensor(out=prod[:], in0=s1[:, 0:OC],
                                            in1=s1[:, OC:2 * OC], op=ALU.mult)
                    a = acc if t < 13 else acc_b
                    if t in (0, 13):
                        nc.vector.tensor_copy(out=a[:], in_=prod[:])
                    else:
                        nc.vector.tensor_tensor(out=a[:], in0=a[:], in1=prod[:],
                                                op=ALU.add)

                kv_dram = []

                def emit_kv(a, tag):
                    ks = wp_.tile([128, OC], F32, tag="kvs")
                    nc.gpsimd.partition_all_reduce(ks[:], a[:], channels=128,
                                                   reduce_op=bass_isa.ReduceOp.add)
                    ki = dp.tile([1, OC], F32, name=f"kvi_{tag}")
                    ko = dp.tile([1, OC], F32, name=f"kvo_{tag}")
                    nc.sync.dma_start(out=ki[:], in_=ks[0:1, :])
                    if collective:
                        nc.gpsimd.collective_compute(
                            "AllReduce", ALU.add,
                            replica_groups=[[0, 1], [2, 3], [4, 5], [6, 7]],
                            ins=[ki[:].opt()], outs=[ko[:].opt()],
                        )
                    else:
                        nc.sync.dma_start(out=ko[:], in_=ki[:])
                    kv_dram.append(ko)

                gbufs = {}
                for t in range(NT):
                    gbufs[t] = emit_d(t)
                    if t >= 3:
                        emit_e(t - 3, gbufs.pop(t - 3))
                    if t == NT - 1:
                        emit_kv(acc, "a")   # tiles 0..12 complete (E(12) emitted)
                for t in range(NT - 3, NT):
                    emit_e(t, gbufs.pop(t))
                emit_kv(acc_b, "b")

            # ================= F: combine kv halves + fold; H: conv chain
            with tc.tile_pool(name="pmz", bufs=3, space="PSUM") as pmz:
                kvr = cp.tile([128, 2], F32)
                kvrb = cp.tile([128, 2], F32)
                nc.sync.dma_start(out=kvr[:],
                                  in_=kv_dram[0][:].rearrange("o (m p) -> o p m", m=2))
                nc.sync.dma_start(out=kvrb[:],
                                  in_=kv_dram[1][:].rearrange("o (m p) -> o p m", m=2))
                nc.vector.tensor_tensor(out=kvr[:], in0=kvr[:], in1=kvrb[:],
                                        op=ALU.add)

                wpk = cp.tile([128, 2, 64], F16)
                for m in range(2):
                    nc.vector.tensor_scalar_mul(out=wpk[:, m, :],
                                                in0=wpt_t[:, m, :],
                                                scalar1=kvr[:, m:m + 1])

                qcs = [qc0, qc1]
                for c in range(ROWS // 512):
                    cs = slice(c * 512, (c + 1) * 512)
                    pp1 = pmz.tile([64, 512], F32, space="PSUM", tag="pp")
                    for m in range(2):
                        nc.tensor.matmul(out=pp1[:], lhsT=wpk[:, m, :],
                                         rhs=qcs[m][:, cs],
                                         start=(m == 0), stop=(m == 1))
                    p1s = wp_.tile([64, 512], F16, tag="p1s")
                    nc.scalar.activation(out=p1s[:], in_=pp1[:], func=AF.Copy)
                    pp2 = pmz.tile([64, 512], F32, space="PSUM", tag="pp")
                    nc.tensor.matmul(out=pp2[:], lhsT=w1t_t[:], rhs=p1s[:],
                                     start=True, stop=True)
                    p2s = wp_.tile([64, 512], F16, tag="p2s")
                    nc.scalar.activation(out=p2s[:], in_=pp2[:], func=AF.Relu,
                                         bias=b1ff_t[:, 0:1])
                    pp3 = pmz.tile([64, 512], F32, space="PSUM", tag="pp")
                    nc.tensor.matmul(out=pp3[:], lhsT=w2t_t[:], rhs=p2s[:],
                                     start=True, stop=True)
                    outs = wp_.tile([64, 512], F32, tag="outs")
                    nc.scalar.activation(out=outs[:], in_=pp3[:], func=AF.Relu,
                                         bias=b2f_t[:, 0:1])
                    (nc.sync if c % 2 == 0 else nc.scalar).dma_start(
                        out=out_half[:, cs], in_=outs[:])

    nc.compile()
    return nc


def _prep_inputs(inputs):
    f = lambda k: np.asarray(inputs[k], dtype=np.float32)
    x = f('x')
    wk, bk = f('wk'), f('bk')
    wq_, bq = f('wq'), f('bq')
    wv, bv = f('wv'), f('bv')
    wp, bp = f('wp'), f('bp')
    w1, b1 = f('w1'), f('b1')
    w2, b2 = f('w2'), f('b2')
    g1, beta1, m1, v1 = f('g1'), f('beta1'), f('m1'), f('v1')
    g2, beta2, m2, v2 = f('g2'), f('beta2'), f('m2'), f('v2')

    s1 = g1 / np.sqrt(v1 + BN_EPS)
    w1f = s1[:, None] * w1
    b1f = s1 * (b1 - m1) + beta1
    s2 = g2 / np.sqrt(v2 + BN_EPS)
    w2f = s2[:, None] * w2
    b2f_v = s2 * (b2 - m2) + beta2
    b1ff = w1f @ bp + b1f  # bp folded through w1f

    wkvq = np.zeros((C + 1, 3 * OC), np.float32)
    wkvq[0:C, 0:OC] = wk.T
    wkvq[C, 0:OC] = bk
    wkvq[0:C, OC:2 * OC] = wv.T
    wkvq[C, OC:2 * OC] = bv
    wkvq[0:C, 2 * OC:] = wq_.T
    wkvq[C, 2 * OC:] = bq
    wpt = np.ascontiguousarray(wp.T.reshape(2, 128, 64).transpose(1, 0, 2)) / 36.0

    shared = {
        "wkvq": wkvq, "wpt": wpt,
        "w1t": np.ascontiguousarray(w1f.T).astype(np.float16),
        "w2t": np.ascontiguousarray(w2f.T).astype(np.float16),
        "b1ff": b1ff.reshape(64, 1), "b2f": b2f_v.reshape(64, 1),
        "one64": np.ones((1, 64), np.float32),
        "i128h": np.eye(128, dtype=np.float16),
        "c36": np.full((128, 1), 1.0 / 36.0, np.float32),
    }
    in_maps = []
    for core in range(8):
        b = core // 2
        roff = (core % 2) * ROWS
        xa = np.ones((C + 1, N), np.float32)
        xa[0:C] = x[b].reshape(C, N)
        m = dict(shared)
        m["xa"] = xa
        m["xr"] = np.ascontiguousarray(xa[:, roff:roff + ROWS])
        xb = x[b].reshape(C, N)
        m["rrow_in"] = (1.0 / np.sqrt((xb * xb).sum(0))).reshape(1, N)
        in_maps.append(m)
    return in_maps


def kernel(**inputs):
    if "nc" not in _CACHE:
        _CACHE["nc"] = _build_bass()
    nc = _CACHE["nc"]
    in_maps = _prep_inputs(inputs)
    res = run_bass_kernel_spmd(nc, in_maps, list(range(8)))
    out = np.empty((B, 64, N), np.float32)
    for core in range(8):
        b = core // 2
        roff = (core % 2) * ROWS
        out[b][:, roff:roff + ROWS] = res.results[core]["out_half"]
    return out.reshape(B, 64, H, W)


if __name__ == "__main__":
    import os
    os.environ.setdefault("JAX_PLATFORMS", "cpu")
    import reference as R
    inputs = R.setup_inputs()
    expected = np.asarray(R.reference(**inputs))
    actual = kernel(**{k: np.asarray(v) for k, v in inputs.items()})
    rel = np.linalg.norm(actual - expected) / np.linalg.norm(expected)
    print("Relative error:", rel)
